# revision 22
# baseline (speedup 1.0000x reference)
"""Trainium2 Bass kernel for nn_ExampleLabelWeights (segment_reduce).

Computes: gather per-example weight rows, masked softmax over each row's
valid slots, weighted sum of losses, global scalar sum.

Strategy (8 NeuronCores, data-parallel over the batch):
  - batch rows (131072) are split 16384/core; per core the (idx, losses)
    pairs are SORTED by idx host-side (the final sum is permutation
    invariant) so the indirect gather walks monotone addresses.
  - the mask is folded into the params table HOST-side: invalid slots are
    set to -100 so exp() gives (effectively) 0 weight. The table is cast
    to bf16 -> 32B rows; ONE indirect-DMA descriptor per batch row.
  - losses are cast to bf16 and packed chunk-major host-side.
  - asymmetric chunks (small first chunk) hide the gather/exp latency.
  - on-device per core, per chunk: indirect gather (GPSIMD SWDGE) ->
    exp on ACT into the lo half of a combo buffer -> nm=ek*lk (DVE mult,
    2x bf16) into the hi half -> ONE fused segmented reduce -> den|num.
    Tail: acc = num/den (one DVE divide), colsum, partition-sum via PE
    matmul with ones, single 4B result DMA straight from PSUM.
  - host sums the 8 per-core scalars.

Written in raw bass (explicit engine programs + semaphores): the walrus
build in this container only supports ONE sync-wait command per
instruction, which TileContext's auto-generated semaphores violate.
"""

from contextlib import ExitStack

import numpy as np
import ml_dtypes

import concourse.bass as bass
import concourse.mybir as mybir
from concourse.bass_utils import run_bass_kernel_spmd

F32 = mybir.dt.float32
BF16 = mybir.dt.bfloat16
I32 = mybir.dt.int32

NCORES = 8
B = 131072
MAXC = 16
V = 1_000_000
P = 128                # SBUF partitions
BC = B // NCORES       # rows per core
COLS = BC // P         # row-groups per partition (128)

CC_LIST = (8, 40, 40, 40)   # per-chunk row-groups per partition
USE_BF16 = True
SORT = True
USE_DIVIDE = False  # divide is not a valid TT ALU op on trn2
MASK_FILL = -100.0     # exp(-100) == 0 in f32/bf16 for all practical purposes


def build_kernel(cc_list=CC_LIST, use_bf16: bool = USE_BF16):
    chunks = len(cc_list)
    assert sum(cc_list) == COLS
    offs = [sum(cc_list[:k]) for k in range(chunks + 1)]
    DT = BF16 if use_bf16 else F32
    nc = bass.Bass()
    ptab = nc.declare_dram_parameter("ptab", [V, MAXC], DT, isOutput=False)
    # idx half A = chunks 0..1, half B = chunks 2..3 (per-chunk column
    # blocks, packed host-side to match idxt's SBUF layout).
    ccA = offs[2]
    ccB = COLS - ccA
    idxA = nc.declare_dram_parameter("idxA", [P, ccA], I32, isOutput=False)
    idxB = nc.declare_dram_parameter("idxB", [P, ccB], I32, isOutput=False)
    lss = [nc.declare_dram_parameter(f"losses{k}", [P, cc_list[k] * MAXC],
                                     DT, isOutput=False)
           for k in range(chunks)]
    out = nc.declare_dram_parameter("out", [1, 1], F32, isOutput=True)

    with ExitStack() as ctx:
        sem_idx = [ctx.enter_context(nc.semaphore(f"sem_idx{h}"))
                   for h in range(2)]
        sem_g = [ctx.enter_context(nc.semaphore(f"sem_g{k}"))
                 for k in range(chunks)]
        sem_l = [ctx.enter_context(nc.semaphore(f"sem_l{k}"))
                 for k in range(chunks)]
        sem_w = ctx.enter_context(nc.semaphore("sem_w"))
        sem_exp = ctx.enter_context(nc.semaphore("sem_exp"))
        sem_dve = ctx.enter_context(nc.semaphore("sem_dve"))
        sem_res = ctx.enter_context(nc.semaphore("sem_res"))
        sem_mm = ctx.enter_context(nc.semaphore("sem_mm"))
        sem_res2 = ctx.enter_context(nc.semaphore("sem_res2"))
        sem_out = ctx.enter_context(nc.semaphore("sem_out"))

        idxt = ctx.enter_context(nc.sbuf_tensor("idxt", [P, COLS], I32))
        widx = ctx.enter_context(nc.sbuf_tensor("widx", [P, 2], I32))
        wbuf = ctx.enter_context(nc.sbuf_tensor("wbuf", [P, 2 * MAXC], DT))
        # dennum viewed [P, 2, COLS]: plane 0 = den, plane 1 = num
        dennum = ctx.enter_context(
            nc.sbuf_tensor("dennum", [P, 2 * COLS], F32))
        rd = ctx.enter_context(nc.sbuf_tensor("rd", [P, COLS], F32))
        acc = ctx.enter_context(nc.sbuf_tensor("acc", [P, COLS], F32))
        colsum = ctx.enter_context(nc.sbuf_tensor("colsum", [P, 1], F32))
        ones = ctx.enter_context(nc.sbuf_tensor("ones", [P, 1], F32))
        scratch = ctx.enter_context(nc.sbuf_tensor("scratch", [P, 1], F32))
        res = ctx.enter_context(nc.sbuf_tensor("res", [1, 1], F32))
        tot = ctx.enter_context(nc.psum_tensor("tot", [1, 1], F32))

        pk, lk, combo = ([] for _ in range(3))
        for k in range(chunks):
            n = cc_list[k] * MAXC
            pk.append(ctx.enter_context(
                nc.sbuf_tensor(f"pk{k}", [P, n], DT)))
            lk.append(ctx.enter_context(
                nc.sbuf_tensor(f"lk{k}", [P, n], DT)))
            # combo: [ek | nm] so one reduce covers both
            combo.append(ctx.enter_context(
                nc.sbuf_tensor(f"combo{k}", [P, 2 * n], DT)))

        def r3(ap, width):
            return ap.rearrange("p (c u) -> p c u", u=width)

        with nc.Block(no_gpsimd_drain=True) as block:

            @block.sync
            def _(sync):
                sync.dma_start(
                    out=idxt[:, 0:ccA], in_=idxA[:, :],
                ).then_inc(sem_idx[0], 16)
                for k in range(2):
                    sync.dma_start(
                        out=lk[k][:, :], in_=lss[k][:, :],
                    ).then_inc(sem_l[k], 16)
                sync.wait_ge(sem_res2, 1)
                sync.dma_start(out=out[:, :], in_=res[:, :]).then_inc(
                    sem_out, 16)
                sync.wait_ge(sem_out, 16)

            @block.gpsimd
            def _(gpsimd):
                # warm up the SWDGE path (Q7 launch + queue start) with a
                # bounds-checked gather over an UNINITIALIZED index tensor:
                # garbage indices > V-1 are silently skipped, in-range ones
                # read harmlessly. No memset/wait needed -> issues at t0.
                gpsimd.indirect_dma_start(
                    out=wbuf[:, :],
                    out_offset=None,
                    in_=ptab[:, :],
                    in_offset=bass.IndirectOffsetOnAxis(
                        ap=widx[:, :], axis=0),
                    bounds_check=V - 1,
                    oob_is_err=False,
                ).then_inc(sem_w, 16)
                for k in range(chunks):
                    gpsimd.wait_ge(sem_idx[0 if offs[k] < ccA else 1], 16)
                    gpsimd.indirect_dma_start(
                        out=pk[k][:, :],
                        out_offset=None,
                        in_=ptab[:, :],
                        in_offset=bass.IndirectOffsetOnAxis(
                            ap=idxt[:, offs[k]:offs[k + 1]], axis=0
                        ),
                    ).then_inc(sem_g[k], 16)

            @block.scalar
            def _(scalar):
                # second idx half + two loss chunks stream from the scalar
                # engine's queue set, in parallel with sync's.
                scalar.dma_start(
                    out=idxt[:, ccA:COLS], in_=idxB[:, :],
                ).then_inc(sem_idx[1], 16)
                # dummy activation: forces the EXP table load off the
                # critical path (overlaps the idx DMA + gather).
                scalar.activation(
                    out=scratch[:, :], in_=scratch[:, :],
                    func=mybir.ActivationFunctionType.Exp,
                )
                for k in range(2, chunks):
                    scalar.dma_start(
                        out=lk[k][:, :], in_=lss[k][:, :],
                    ).then_inc(sem_l[k], 16)
                for k in range(chunks):
                    n = cc_list[k] * MAXC
                    scalar.wait_ge(sem_g[k], 16)
                    scalar.activation(
                        out=combo[k][:, 0:n], in_=pk[k][:, :],
                        func=mybir.ActivationFunctionType.Exp,
                    ).then_inc(sem_exp, 1)

            # The DVE pipeline does not interlock same-engine RAW hazards:
            # every dependent pair needs an explicit wait on the engine's
            # completion counter.
            @block.vector
            def _(vector):
                state = {"n": 0, "hw": 0}

                def bump(inst):
                    state["n"] += 1
                    inst.then_inc(sem_dve, 1)
                    return state["n"]

                def dep(*ths):
                    th = max(ths)
                    if th > state["hw"]:
                        vector.wait_ge(sem_dve, th)
                        state["hw"] = th

                bump(vector.memset(ones[:, :], 1.0))
                dn_v = r3(dennum[:, :], COLS)  # [P, 2, COLS]
                i_red = [0] * chunks
                for k in range(chunks):
                    n = cc_list[k] * MAXC
                    vector.wait_ge(sem_exp, k + 1)
                    vector.wait_ge(sem_l[k], 16)
                    i_mult = bump(vector.tensor_tensor(
                        out=combo[k][:, n:2 * n],
                        in0=combo[k][:, 0:n], in1=lk[k][:, :],
                        op=mybir.AluOpType.mult,
                    ))
                    dep(i_mult)
                    # fused reduce emits [den_k | num_k]; the strided out AP
                    # routes them into the den/num planes.
                    i_red[k] = bump(vector.tensor_reduce(
                        out=dn_v[:, :, offs[k]:offs[k + 1]],
                        in_=r3(combo[k][:, :], MAXC)[:, :, :],
                        axis=mybir.AxisListType.X,
                        op=mybir.AluOpType.add,
                    ))
                dep(*i_red)
                i_rd = bump(vector.reciprocal(
                    out=rd[:, :], in_=dn_v[:, 0, :]))
                dep(i_rd)
                i_acc = bump(vector.tensor_tensor(
                    out=acc[:, :], in0=dn_v[:, 1, :], in1=rd[:, :],
                    op=mybir.AluOpType.mult,
                ))
                dep(i_acc)
                vector.tensor_reduce(
                    out=colsum[:, :], in_=acc[:, :],
                    axis=mybir.AxisListType.X, op=mybir.AluOpType.add,
                ).then_inc(sem_res, 1)
                vector.wait_ge(sem_mm, 1)
                vector.tensor_copy(out=res[:, :], in_=tot[:, :]).then_inc(
                    sem_res2, 1)

            @block.tensor
            def _(tensor):
                tensor.wait_ge(sem_res, 1)
                tensor.matmul(
                    out=tot[:, :], lhsT=colsum[:, :], rhs=ones[:, :],
                    start=True, stop=True,
                ).then_inc(sem_mm, 1)

    return nc


def make_inputs(losses, inputs_idx, params, cardinality,
                cc_list=CC_LIST, use_bf16: bool = USE_BF16,
                sort: bool = SORT):
    """Pack/shard full inputs into per-core input maps."""
    npdt = ml_dtypes.bfloat16 if use_bf16 else np.float32
    chunks = len(cc_list)
    offs = [sum(cc_list[:k]) for k in range(chunks + 1)]
    ccA = offs[2]
    p = np.asarray(params, dtype=np.float32)
    card = np.asarray(cardinality, dtype=np.int32)
    mask = np.arange(MAXC, dtype=np.int32)[None, :] < card[:, None]
    ptab = np.where(mask, p, np.float32(MASK_FILL)).astype(npdt)
    idx_full = np.asarray(inputs_idx, dtype=np.int32)
    losses_full = np.asarray(losses, dtype=np.float32)
    in_maps = []
    for c in range(NCORES):
        sl = slice(c * BC, (c + 1) * BC)
        idx_c = idx_full[sl]
        losses_c = losses_full[sl]
        if sort:
            order = np.argsort(idx_c)
            idx_c = idx_c[order]
            losses_c = losses_c[order]
        # chunk k holds sorted ranks [offs[k]*P, offs[k+1]*P), laid out
        # [P, cc_k] row-major; on SBUF it sits at idxt[:, offs_k:offs_k+1].
        idx_cols = [idx_c[offs[k] * P:offs[k + 1] * P].reshape(P, cc_list[k])
                    for k in range(chunks)]
        m = {
            "ptab": ptab,
            "idxA": np.ascontiguousarray(
                np.concatenate(idx_cols[:2], axis=1)),
            "idxB": np.ascontiguousarray(
                np.concatenate(idx_cols[2:], axis=1)),
        }
        for k in range(chunks):
            m[f"losses{k}"] = np.ascontiguousarray(
                losses_c[offs[k] * P:offs[k + 1] * P]
                .reshape(P, cc_list[k] * MAXC).astype(npdt))
        in_maps.append(m)
    return in_maps


_NC_CACHE = {}


def kernel(losses, inputs_idx, params, cardinality, trace=False, **kw):
    key = (CC_LIST, USE_BF16)
    if key not in _NC_CACHE:
        _NC_CACHE[key] = build_kernel(CC_LIST, USE_BF16)
    nc = _NC_CACHE[key]
    in_maps = make_inputs(losses, inputs_idx, params, cardinality,
                          CC_LIST, USE_BF16, SORT)
    r = run_bass_kernel_spmd(nc, in_maps, list(range(NCORES)), trace=trace, **kw)
    total = np.float64(0.0)
    for c in range(NCORES):
        total += np.float64(np.asarray(r.results[c]["out"],
                                       dtype=np.float32).sum(dtype=np.float64))
    out = np.float32(total)
    if trace:
        kernel.last_results = r
    return np.asarray(out)


kernel.last_results = None


# revision 24
# speedup vs baseline: 1.0445x; 1.0445x over previous
"""Trainium2 Bass kernel for nn_ExampleLabelWeights (segment_reduce).

Computes: gather per-example weight rows, masked softmax over each row's
valid slots, weighted sum of losses, global scalar sum.

Strategy (8 NeuronCores, data-parallel over the batch):
  - batch rows (131072) are split 16384/core; per core the (idx, losses)
    pairs are SORTED by idx host-side (the final sum is permutation
    invariant) so the indirect gather walks monotone addresses.
  - the mask is folded into the params table HOST-side: invalid slots are
    set to -100 so exp() gives (effectively) 0 weight. The table is cast
    to bf16 -> 32B rows; ONE indirect-DMA descriptor per batch row.
  - the SWDGE reads the offset lists straight from DRAM (no idx staging
    DMA); losses are cast to bf16 and streamed chunk-major.
  - on-device per core, per chunk: indirect gather (GPSIMD SWDGE) ->
    exp on ACT into the lo half of a combo buffer -> nm=ek*lk (DVE mult,
    2x bf16) into the hi half -> ONE fused segmented reduce whose strided
    out AP routes den/num into separate planes. Tail: reciprocal + ratio
    + colsum (DVE), partition-sum via PE matmul, single 4B DMA out.
  - host sums the 8 per-core scalars.

Written in raw bass (explicit engine programs + semaphores): the walrus
build in this container only supports ONE sync-wait command per
instruction, which TileContext's auto-generated semaphores violate.
"""

from contextlib import ExitStack

import numpy as np
import ml_dtypes

import concourse.bass as bass
import concourse.mybir as mybir
from concourse.bass_utils import run_bass_kernel_spmd

F32 = mybir.dt.float32
BF16 = mybir.dt.bfloat16
I32 = mybir.dt.int32

NCORES = 8
B = 131072
MAXC = 16
V = 1_000_000
P = 128                # SBUF partitions
BC = B // NCORES       # rows per core
COLS = BC // P         # row-groups per partition (128)

CC_LIST = (32, 32, 32, 32)  # per-chunk row-groups per partition
USE_BF16 = True
SORT = True
DRAM_OFFSETS = False   # walrus rejects DRAM offset APs; stage idx in SBUF
MASK_FILL = -100.0     # exp(-100) == 0 in f32/bf16 for all practical purposes


def build_kernel(cc_list=CC_LIST, use_bf16: bool = USE_BF16,
                 dram_offsets: bool = DRAM_OFFSETS):
    chunks = len(cc_list)
    assert sum(cc_list) == COLS
    offs = [sum(cc_list[:k]) for k in range(chunks + 1)]
    DT = BF16 if use_bf16 else F32
    nc = bass.Bass()
    ptab = nc.declare_dram_parameter("ptab", [V, MAXC], DT, isOutput=False)
    ccA = offs[2]
    idx = nc.declare_dram_parameter("idx", [P, COLS], I32, isOutput=False)
    lss = [nc.declare_dram_parameter(f"losses{k}", [P, cc_list[k] * MAXC],
                                     DT, isOutput=False)
           for k in range(chunks)]
    out = nc.declare_dram_parameter("out", [1, 1], F32, isOutput=True)

    with ExitStack() as ctx:
        sem_idx = [ctx.enter_context(nc.semaphore(f"sem_idx{h}"))
                   for h in range(2)]
        sem_g = [ctx.enter_context(nc.semaphore(f"sem_g{k}"))
                 for k in range(chunks)]
        sem_l = [ctx.enter_context(nc.semaphore(f"sem_l{k}"))
                 for k in range(chunks)]
        sem_w = ctx.enter_context(nc.semaphore("sem_w"))
        sem_exp = ctx.enter_context(nc.semaphore("sem_exp"))
        sem_dve = ctx.enter_context(nc.semaphore("sem_dve"))
        sem_res = ctx.enter_context(nc.semaphore("sem_res"))
        sem_mm = ctx.enter_context(nc.semaphore("sem_mm"))
        sem_res2 = ctx.enter_context(nc.semaphore("sem_res2"))
        sem_out = ctx.enter_context(nc.semaphore("sem_out"))

        idxt = ctx.enter_context(nc.sbuf_tensor("idxt", [P, COLS], I32))
        widx = ctx.enter_context(nc.sbuf_tensor("widx", [P, 2], I32))
        wbuf = ctx.enter_context(nc.sbuf_tensor("wbuf", [P, 2 * MAXC], DT))
        # dennum viewed [P, 2, COLS]: plane 0 = den, plane 1 = num
        dennum = ctx.enter_context(
            nc.sbuf_tensor("dennum", [P, 2 * COLS], F32))
        rd = ctx.enter_context(nc.sbuf_tensor("rd", [P, COLS], F32))
        acc = ctx.enter_context(nc.sbuf_tensor("acc", [P, COLS], F32))
        colsum = ctx.enter_context(nc.sbuf_tensor("colsum", [P, 1], F32))
        ones = ctx.enter_context(nc.sbuf_tensor("ones", [P, 1], F32))
        scratch = ctx.enter_context(nc.sbuf_tensor("scratch", [P, 1], F32))
        res = ctx.enter_context(nc.sbuf_tensor("res", [1, 1], F32))
        tot = ctx.enter_context(nc.psum_tensor("tot", [1, 1], F32))

        pk, lk, combo = ([] for _ in range(3))
        for k in range(chunks):
            n = cc_list[k] * MAXC
            pk.append(ctx.enter_context(
                nc.sbuf_tensor(f"pk{k}", [P, n], DT)))
            lk.append(ctx.enter_context(
                nc.sbuf_tensor(f"lk{k}", [P, n], DT)))
            # combo: [ek | nm] so one reduce covers both
            combo.append(ctx.enter_context(
                nc.sbuf_tensor(f"combo{k}", [P, 2 * n], DT)))

        def r3(ap, width):
            return ap.rearrange("p (c u) -> p c u", u=width)

        with nc.Block(no_gpsimd_drain=True) as block:

            @block.sync
            def _(sync):
                if not dram_offsets:
                    sync.dma_start(
                        out=idxt[:, 0:ccA], in_=idx[:, 0:ccA],
                    ).then_inc(sem_idx[0], 16)
                for k in range(2):
                    sync.dma_start(
                        out=lk[k][:, :], in_=lss[k][:, :],
                    ).then_inc(sem_l[k], 16)
                sync.wait_ge(sem_res2, 1)
                sync.dma_start(out=out[:, :], in_=res[:, :]).then_inc(
                    sem_out, 16)
                sync.wait_ge(sem_out, 16)

            @block.gpsimd
            def _(gpsimd):
                # warm up the SWDGE path (Q7 launch + queue start) with a
                # bounds-checked gather over an UNINITIALIZED index tensor:
                # garbage indices > V-1 are silently skipped, in-range ones
                # read harmlessly. No memset/wait needed -> issues at t0.
                gpsimd.indirect_dma_start(
                    out=wbuf[:, :],
                    out_offset=None,
                    in_=ptab[:, :],
                    in_offset=bass.IndirectOffsetOnAxis(
                        ap=widx[:, :], axis=0),
                    bounds_check=V - 1,
                    oob_is_err=False,
                ).then_inc(sem_w, 16)
                for k in range(chunks):
                    if dram_offsets:
                        off_ap = idx[:, offs[k]:offs[k + 1]]
                    else:
                        gpsimd.wait_ge(
                            sem_idx[0 if offs[k] < ccA else 1], 16)
                        off_ap = idxt[:, offs[k]:offs[k + 1]]
                    gpsimd.indirect_dma_start(
                        out=pk[k][:, :],
                        out_offset=None,
                        in_=ptab[:, :],
                        in_offset=bass.IndirectOffsetOnAxis(
                            ap=off_ap, axis=0),
                    ).then_inc(sem_g[k], 16)

            @block.scalar
            def _(scalar):
                if not dram_offsets:
                    scalar.dma_start(
                        out=idxt[:, ccA:COLS], in_=idx[:, ccA:COLS],
                    ).then_inc(sem_idx[1], 16)
                # dummy activation: forces the EXP table load off the
                # critical path (overlaps the gather).
                scalar.activation(
                    out=scratch[:, :], in_=scratch[:, :],
                    func=mybir.ActivationFunctionType.Exp,
                )
                for k in range(2, chunks):
                    scalar.dma_start(
                        out=lk[k][:, :], in_=lss[k][:, :],
                    ).then_inc(sem_l[k], 16)
                for k in range(chunks):
                    n = cc_list[k] * MAXC
                    scalar.wait_ge(sem_g[k], 16)
                    scalar.activation(
                        out=combo[k][:, 0:n], in_=pk[k][:, :],
                        func=mybir.ActivationFunctionType.Exp,
                    ).then_inc(sem_exp, 1)

            # The DVE pipeline does not interlock same-engine RAW hazards:
            # every dependent pair needs an explicit wait on the engine's
            # completion counter.
            @block.vector
            def _(vector):
                state = {"n": 0, "hw": 0}

                def bump(inst):
                    state["n"] += 1
                    inst.then_inc(sem_dve, 1)
                    return state["n"]

                def dep(*ths):
                    th = max(ths)
                    if th > state["hw"]:
                        vector.wait_ge(sem_dve, th)
                        state["hw"] = th

                bump(vector.memset(ones[:, :], 1.0))
                dn_v = r3(dennum[:, :], COLS)  # [P, 2, COLS]
                i_red = [0] * chunks
                for k in range(chunks):
                    n = cc_list[k] * MAXC
                    vector.wait_ge(sem_exp, k + 1)
                    vector.wait_ge(sem_l[k], 16)
                    i_mult = bump(vector.tensor_tensor(
                        out=combo[k][:, n:2 * n],
                        in0=combo[k][:, 0:n], in1=lk[k][:, :],
                        op=mybir.AluOpType.mult,
                    ))
                    dep(i_mult)
                    # fused reduce emits [den_k | num_k]; the strided out AP
                    # routes them into the den/num planes.
                    i_red[k] = bump(vector.tensor_reduce(
                        out=dn_v[:, :, offs[k]:offs[k + 1]],
                        in_=r3(combo[k][:, :], MAXC)[:, :, :],
                        axis=mybir.AxisListType.X,
                        op=mybir.AluOpType.add,
                    ))
                dep(*i_red)
                i_rd = bump(vector.reciprocal(
                    out=rd[:, :], in_=dn_v[:, 0, :]))
                dep(i_rd)
                i_acc = bump(vector.tensor_tensor(
                    out=acc[:, :], in0=dn_v[:, 1, :], in1=rd[:, :],
                    op=mybir.AluOpType.mult,
                ))
                dep(i_acc)
                vector.tensor_reduce(
                    out=colsum[:, :], in_=acc[:, :],
                    axis=mybir.AxisListType.X, op=mybir.AluOpType.add,
                ).then_inc(sem_res, 1)
                vector.wait_ge(sem_mm, 1)
                vector.tensor_copy(out=res[:, :], in_=tot[:, :]).then_inc(
                    sem_res2, 1)

            @block.tensor
            def _(tensor):
                tensor.wait_ge(sem_res, 1)
                tensor.matmul(
                    out=tot[:, :], lhsT=colsum[:, :], rhs=ones[:, :],
                    start=True, stop=True,
                ).then_inc(sem_mm, 1)

    return nc


def make_inputs(losses, inputs_idx, params, cardinality,
                cc_list=CC_LIST, use_bf16: bool = USE_BF16,
                sort: bool = SORT):
    """Pack/shard full inputs into per-core input maps."""
    npdt = ml_dtypes.bfloat16 if use_bf16 else np.float32
    chunks = len(cc_list)
    offs = [sum(cc_list[:k]) for k in range(chunks + 1)]
    p = np.asarray(params, dtype=np.float32)
    card = np.asarray(cardinality, dtype=np.int32)
    mask = np.arange(MAXC, dtype=np.int32)[None, :] < card[:, None]
    ptab = np.where(mask, p, np.float32(MASK_FILL)).astype(npdt)
    idx_full = np.asarray(inputs_idx, dtype=np.int32)
    losses_full = np.asarray(losses, dtype=np.float32)
    in_maps = []
    for c in range(NCORES):
        sl = slice(c * BC, (c + 1) * BC)
        idx_c = idx_full[sl]
        losses_c = losses_full[sl]
        if sort:
            order = np.argsort(idx_c)
            idx_c = idx_c[order]
            losses_c = losses_c[order]
        # chunk k holds sorted ranks [offs[k]*P, offs[k+1]*P), laid out
        # [P, cc_k] row-major; it sits at idx[:, offs_k:offs_k+1].
        idx_cols = [idx_c[offs[k] * P:offs[k + 1] * P].reshape(P, cc_list[k])
                    for k in range(chunks)]
        m = {
            "ptab": ptab,
            "idx": np.ascontiguousarray(np.concatenate(idx_cols, axis=1)),
        }
        for k in range(chunks):
            m[f"losses{k}"] = np.ascontiguousarray(
                losses_c[offs[k] * P:offs[k + 1] * P]
                .reshape(P, cc_list[k] * MAXC).astype(npdt))
        in_maps.append(m)
    return in_maps


_NC_CACHE = {}


def kernel(losses, inputs_idx, params, cardinality, trace=False, **kw):
    key = (CC_LIST, USE_BF16, DRAM_OFFSETS)
    if key not in _NC_CACHE:
        _NC_CACHE[key] = build_kernel(CC_LIST, USE_BF16, DRAM_OFFSETS)
    nc = _NC_CACHE[key]
    in_maps = make_inputs(losses, inputs_idx, params, cardinality,
                          CC_LIST, USE_BF16, SORT)
    r = run_bass_kernel_spmd(nc, in_maps, list(range(NCORES)), trace=trace, **kw)
    total = np.float64(0.0)
    for c in range(NCORES):
        total += np.float64(np.asarray(r.results[c]["out"],
                                       dtype=np.float32).sum(dtype=np.float64))
    out = np.float32(total)
    if trace:
        kernel.last_results = r
    return np.asarray(out)


kernel.last_results = None


# revision 30
# speedup vs baseline: 1.0819x; 1.0358x over previous
"""Trainium2 Bass kernel for nn_ExampleLabelWeights (segment_reduce).

Computes: gather per-example weight rows, masked softmax over each row's
valid slots, weighted sum of losses, global scalar sum.

Strategy (8 NeuronCores, data-parallel over the batch):
  - batch rows (131072) are split 16384/core; per core the rows are
    SORTED host-side by (cardinality, idx) — the final sum is permutation
    invariant. Sorting by cardinality lets each quarter-chunk use a
    NARROW slot width (low-card rows only need their first few slots:
    the rest are masked padding), cutting gather bytes, exp work and DVE
    work to ~70%. Sorting by idx within a cardinality class keeps the
    indirect gather walking near-monotone addresses.
  - the mask is folded into the params table HOST-side: invalid slots are
    set to -100 so exp() gives (effectively) 0 weight. The table is cast
    to bf16 -> 32B rows; ONE indirect-DMA descriptor per batch row reads
    just the chunk's slot width.
  - per-chunk slot widths are derived from the data at first call
    (rounded up to even); the compiled NEFF is cached per width tuple.
  - on-device per core, per chunk: indirect gather (GPSIMD SWDGE) ->
    exp on ACT into the lo half of a combo buffer -> nm=ek*lk (DVE mult,
    2x bf16) into the hi half -> ONE fused segmented reduce whose strided
    out AP routes den/num into separate planes. Tail: reciprocal + ratio
    + colsum (DVE), partition-sum via PE matmul, single 4B DMA out.
  - host sums the 8 per-core scalars.

Written in raw bass (explicit engine programs + semaphores): the walrus
build in this container only supports ONE sync-wait command per
instruction, which TileContext's auto-generated semaphores violate.
"""

from contextlib import ExitStack

import numpy as np
import ml_dtypes

import concourse.bass as bass
import concourse.mybir as mybir
from concourse.bass_utils import run_bass_kernel_spmd

F32 = mybir.dt.float32
BF16 = mybir.dt.bfloat16
I32 = mybir.dt.int32

NCORES = 8
B = 131072
MAXC = 16
V = 1_000_000
P = 128                # SBUF partitions
BC = B // NCORES       # rows per core
COLS = BC // P         # row-groups per partition (128)

CC_LIST = (32, 32, 32, 32)  # per-chunk row-groups per partition
USE_BF16 = True
MASK_FILL = -100.0     # exp(-100) == 0 in f32/bf16 for all practical purposes


def build_kernel(widths, cc_list=CC_LIST, use_bf16: bool = USE_BF16):
    chunks = len(cc_list)
    assert sum(cc_list) == COLS
    offs = [sum(cc_list[:k]) for k in range(chunks + 1)]
    DT = BF16 if use_bf16 else F32
    nc = bass.Bass()
    # one packed table per distinct width: physical row pitch == width
    # (the indirect DMA computes src addr as index * row_size, so the
    # table must be contiguous at that width).
    ptabs = {w: nc.declare_dram_parameter(f"ptab{w}", [V, w], DT,
                                          isOutput=False)
             for w in sorted(set(widths))}
    cc0 = cc_list[0]
    idx0 = nc.declare_dram_parameter("idx0", [P, cc0], I32, isOutput=False)
    idxR = nc.declare_dram_parameter("idxR", [P, COLS - cc0], I32,
                                     isOutput=False)
    lss = [nc.declare_dram_parameter(
        f"losses{k}", [P, cc_list[k] * widths[k]], DT, isOutput=False)
        for k in range(chunks)]
    out = nc.declare_dram_parameter("out", [1, 1], F32, isOutput=True)

    with ExitStack() as ctx:
        sem_idx = [ctx.enter_context(nc.semaphore(f"sem_idx{h}"))
                   for h in range(2)]
        sem_g = [ctx.enter_context(nc.semaphore(f"sem_g{k}"))
                 for k in range(chunks)]
        sem_l = [ctx.enter_context(nc.semaphore(f"sem_l{k}"))
                 for k in range(chunks)]
        sem_w = ctx.enter_context(nc.semaphore("sem_w"))
        sem_exp = ctx.enter_context(nc.semaphore("sem_exp"))
        sem_dve = ctx.enter_context(nc.semaphore("sem_dve"))
        sem_res = ctx.enter_context(nc.semaphore("sem_res"))
        sem_mm = ctx.enter_context(nc.semaphore("sem_mm"))
        sem_res2 = ctx.enter_context(nc.semaphore("sem_res2"))
        sem_out = ctx.enter_context(nc.semaphore("sem_out"))

        idxt = ctx.enter_context(nc.sbuf_tensor("idxt", [P, COLS], I32))
        widx = ctx.enter_context(nc.sbuf_tensor("widx", [P, 8], I32))
        wbuf = ctx.enter_context(nc.sbuf_tensor("wbuf", [P, 8 * MAXC], DT))
        # dennum viewed [P, 2, COLS]: plane 0 = den, plane 1 = num
        dennum = ctx.enter_context(
            nc.sbuf_tensor("dennum", [P, 2 * COLS], F32))
        rd = ctx.enter_context(nc.sbuf_tensor("rd", [P, COLS], F32))
        acc = ctx.enter_context(nc.sbuf_tensor("acc", [P, COLS], F32))
        colsum = ctx.enter_context(nc.sbuf_tensor("colsum", [P, 1], F32))
        ones = ctx.enter_context(nc.sbuf_tensor("ones", [P, 1], F32))
        scratch = ctx.enter_context(nc.sbuf_tensor("scratch", [P, 1], F32))
        res = ctx.enter_context(nc.sbuf_tensor("res", [1, 1], F32))
        tot = ctx.enter_context(nc.psum_tensor("tot", [1, 1], F32))

        pk, lk, combo = ([] for _ in range(3))
        for k in range(chunks):
            n = cc_list[k] * widths[k]
            pk.append(ctx.enter_context(
                nc.sbuf_tensor(f"pk{k}", [P, n], DT)))
            lk.append(ctx.enter_context(
                nc.sbuf_tensor(f"lk{k}", [P, n], DT)))
            # combo: [ek | nm] so one reduce covers both
            combo.append(ctx.enter_context(
                nc.sbuf_tensor(f"combo{k}", [P, 2 * n], DT)))

        def r3(ap, width):
            return ap.rearrange("p (c u) -> p c u", u=width)

        with nc.Block(no_gpsimd_drain=True) as block:

            @block.sync
            def _(sync):
                sync.dma_start(
                    out=idxt[:, 0:cc0], in_=idx0[:, :],
                ).then_inc(sem_idx[0], 16)
                for k in range(2):
                    sync.dma_start(
                        out=lk[k][:, :], in_=lss[k][:, :],
                    ).then_inc(sem_l[k], 16)
                sync.wait_ge(sem_res2, 1)
                sync.dma_start(out=out[:, :], in_=res[:, :]).then_inc(
                    sem_out, 16)
                sync.wait_ge(sem_out, 16)

            @block.gpsimd
            def _(gpsimd):
                # warm up the SWDGE path (Q7 launch + queue start) with a
                # bounds-checked gather over an UNINITIALIZED index tensor:
                # garbage indices > V-1 are silently skipped, in-range ones
                # read harmlessly. No memset/wait needed -> issues at t0.
                wtab = ptabs[widths[0]]
                gpsimd.indirect_dma_start(
                    out=wbuf[:, 0:8 * widths[0]],
                    out_offset=None,
                    in_=wtab[:, :],
                    in_offset=bass.IndirectOffsetOnAxis(
                        ap=widx[:, :], axis=0),
                    bounds_check=V - 1,
                    oob_is_err=False,
                ).then_inc(sem_w, 16)
                for k in range(chunks):
                    gpsimd.wait_ge(sem_idx[0 if k == 0 else 1], 16)
                    gpsimd.indirect_dma_start(
                        out=pk[k][:, :],
                        out_offset=None,
                        in_=ptabs[widths[k]][:, :],
                        in_offset=bass.IndirectOffsetOnAxis(
                            ap=idxt[:, offs[k]:offs[k + 1]], axis=0
                        ),
                    ).then_inc(sem_g[k], 16)

            @block.scalar
            def _(scalar):
                scalar.dma_start(
                    out=idxt[:, cc0:COLS], in_=idxR[:, :],
                ).then_inc(sem_idx[1], 16)
                # dummy activation: forces the EXP table load off the
                # critical path (overlaps the gather).
                scalar.activation(
                    out=scratch[:, :], in_=scratch[:, :],
                    func=mybir.ActivationFunctionType.Exp,
                )
                for k in range(2, chunks):
                    scalar.dma_start(
                        out=lk[k][:, :], in_=lss[k][:, :],
                    ).then_inc(sem_l[k], 16)
                for k in range(chunks):
                    n = cc_list[k] * widths[k]
                    scalar.wait_ge(sem_g[k], 16)
                    scalar.activation(
                        out=combo[k][:, 0:n], in_=pk[k][:, :],
                        func=mybir.ActivationFunctionType.Exp,
                    ).then_inc(sem_exp, 1)

            # The DVE pipeline does not interlock same-engine RAW hazards:
            # every dependent pair needs an explicit wait on the engine's
            # completion counter.
            @block.vector
            def _(vector):
                state = {"n": 0, "hw": 0}

                def bump(inst):
                    state["n"] += 1
                    inst.then_inc(sem_dve, 1)
                    return state["n"]

                def dep(*ths):
                    th = max(ths)
                    if th > state["hw"]:
                        vector.wait_ge(sem_dve, th)
                        state["hw"] = th

                bump(vector.memset(ones[:, :], 1.0))
                dn_v = r3(dennum[:, :], COLS)  # [P, 2, COLS]
                i_red = [0] * chunks
                for k in range(chunks):
                    n = cc_list[k] * widths[k]
                    vector.wait_ge(sem_exp, k + 1)
                    vector.wait_ge(sem_l[k], 16)
                    i_mult = bump(vector.tensor_tensor(
                        out=combo[k][:, n:2 * n],
                        in0=combo[k][:, 0:n], in1=lk[k][:, :],
                        op=mybir.AluOpType.mult,
                    ))
                    dep(i_mult)
                    # fused reduce emits [den_k | num_k]; the strided out AP
                    # routes them into the den/num planes.
                    i_red[k] = bump(vector.tensor_reduce(
                        out=dn_v[:, :, offs[k]:offs[k + 1]],
                        in_=r3(combo[k][:, :], widths[k])[:, :, :],
                        axis=mybir.AxisListType.X,
                        op=mybir.AluOpType.add,
                    ))
                dep(*i_red)
                i_rd = bump(vector.reciprocal(
                    out=rd[:, :], in_=dn_v[:, 0, :]))
                dep(i_rd)
                i_acc = bump(vector.tensor_tensor(
                    out=acc[:, :], in0=dn_v[:, 1, :], in1=rd[:, :],
                    op=mybir.AluOpType.mult,
                ))
                dep(i_acc)
                vector.tensor_reduce(
                    out=colsum[:, :], in_=acc[:, :],
                    axis=mybir.AxisListType.X, op=mybir.AluOpType.add,
                ).then_inc(sem_res, 1)
                vector.wait_ge(sem_mm, 1)
                vector.tensor_copy(out=res[:, :], in_=tot[:, :]).then_inc(
                    sem_res2, 1)

            @block.tensor
            def _(tensor):
                tensor.wait_ge(sem_res, 1)
                tensor.matmul(
                    out=tot[:, :], lhsT=colsum[:, :], rhs=ones[:, :],
                    start=True, stop=True,
                ).then_inc(sem_mm, 1)

    return nc


def make_inputs(losses, inputs_idx, params, cardinality,
                cc_list=CC_LIST, use_bf16: bool = USE_BF16):
    """Pack/shard full inputs into per-core input maps.

    Returns (in_maps, widths): rows sorted by (cardinality, idx); chunk k
    spans sorted ranks [offs[k]*P, offs[k+1]*P) with slot width
    widths[k] = even-rounded max cardinality in that block (over cores).
    """
    npdt = ml_dtypes.bfloat16 if use_bf16 else np.float32
    chunks = len(cc_list)
    offs = [sum(cc_list[:k]) for k in range(chunks + 1)]
    cc0 = cc_list[0]
    p = np.asarray(params, dtype=np.float32)
    card = np.asarray(cardinality, dtype=np.int32)
    mask = np.arange(MAXC, dtype=np.int32)[None, :] < card[:, None]
    idx_full = np.asarray(inputs_idx, dtype=np.int32)
    losses_full = np.asarray(losses, dtype=np.float32)

    per_core = []
    widths = [2] * chunks
    for c in range(NCORES):
        sl = slice(c * BC, (c + 1) * BC)
        idx_c = idx_full[sl]
        losses_c = losses_full[sl]
        card_c = card[idx_c]
        order = np.lexsort((idx_c, card_c))
        idx_c = idx_c[order]
        losses_c = losses_c[order]
        card_c = card_c[order]
        per_core.append((idx_c, losses_c))
        for k in range(chunks):
            mx = int(card_c[offs[k] * P:offs[k + 1] * P].max())
            widths[k] = max(widths[k], (mx + 1) // 2 * 2)
    widths = tuple(widths)

    ptabs = {w: np.ascontiguousarray(
        np.where(mask[:, :w], p[:, :w], np.float32(MASK_FILL)).astype(npdt))
        for w in sorted(set(widths))}
    in_maps = []
    for idx_c, losses_c in per_core:
        idx_cols = [idx_c[offs[k] * P:offs[k + 1] * P].reshape(P, cc_list[k])
                    for k in range(chunks)]
        m = {f"ptab{w}": t for w, t in ptabs.items()}
        m.update({
            "idx0": np.ascontiguousarray(idx_cols[0]),
            "idxR": np.ascontiguousarray(
                np.concatenate(idx_cols[1:], axis=1)),
        })
        for k in range(chunks):
            w = widths[k]
            blk = losses_c[offs[k] * P:offs[k + 1] * P, 0:w]
            m[f"losses{k}"] = np.ascontiguousarray(
                blk.reshape(P, cc_list[k] * w).astype(npdt))
        in_maps.append(m)
    return in_maps, widths


_NC_CACHE = {}


def kernel(losses, inputs_idx, params, cardinality, trace=False, **kw):
    in_maps, widths = make_inputs(losses, inputs_idx, params, cardinality,
                                  CC_LIST, USE_BF16)
    key = (CC_LIST, USE_BF16, widths)
    if key not in _NC_CACHE:
        _NC_CACHE[key] = build_kernel(widths, CC_LIST, USE_BF16)
    nc = _NC_CACHE[key]
    r = run_bass_kernel_spmd(nc, in_maps, list(range(NCORES)), trace=trace, **kw)
    total = np.float64(0.0)
    for c in range(NCORES):
        total += np.float64(np.asarray(r.results[c]["out"],
                                       dtype=np.float32).sum(dtype=np.float64))
    out = np.float32(total)
    if trace:
        kernel.last_results = r
    return np.asarray(out)


kernel.last_results = None
